# revision 25
# baseline (speedup 1.0000x reference)
"""Trainium2 Bass kernel for nn_Encoder_81595788689580.

Attention-gated GRU encoder: per time step
    w1 = h @ attn1_W.T + attn1_b
    w2 = x_t @ attn2_W.T + attn2_b
    v  = tanh(w1 + w2) @ attn3_W.T + attn3_b
    alpha = softmax(v, axis=feature)
    wx = x_t * alpha
    GRU cell (r, z, n) -> h_new
Output: [B, T, H] hidden states.

Strategy (8 NeuronCores, data-parallel over batch):
  - batch 4096 -> 512 rows per core; all weights replicated.
  - everything stored TRANSPOSED on chip: features on partitions, batch on
    the free dim. Every matmul is weights-stationary with batch as the
    moving dim, biases become per-partition ACT bias vectors, and no
    transposes are ever needed on device (host pre-/post-transposes).
  - feature dim I=320 zero-padded to 384 = 3x128 partition blocks; padded
    attn3_b rows are -1e4 so exp() of pad rows is exactly 0 and the
    softmax denominator is unaffected.
  - softmax over features is a partition reduction: an all-ones stationary
    matmul broadcasts the per-column denominator into all 128 partitions
    of one PSUM tile; max-subtraction is skipped (|v| <= ~8 in practice,
    exp stays finite, softmax is shift-invariant).
  - sigmoid is computed as 0.5*tanh(x/2)+0.5 so every ACT op uses the
    exp_and_others table set -- avoids ~2.7us ACT table swaps per step.
  - matmuls in fp16 (1 PE cycle/row, fast weight loads) with fp32 PSUM
    accumulation; exp output in bf16 (range safety). DT="f32r" switches
    to float32r matmuls (~10x lower error, ~1.7x slower weight loads).
  - the 512-row batch runs as 2 independent chunks of 256 so the two
    recurrences pipeline against each other across engines.
"""

import numpy as np

B, T, I, H = 4096, 24, 320, 256
NCORES = 8
BS = B // NCORES          # 512 rows per core
IP = 384                  # I padded to 3*128
KI = IP // 128            # 3 feature blocks
KH = H // 128             # 2 hidden blocks
G = 3 * H                 # 768 gate rows
NCHUNK = 2
CB = BS // NCHUNK         # 256 batch columns per chunk

DT = "f16"                # "f16" | "f32r"

_STATE = {}


def _np_dt(mdt):
    from concourse import mybir
    return mybir.dt.np(mdt)


def _dts():
    from concourse import mybir
    if DT == "f16":
        return mybir.dt.float16, mybir.dt.bfloat16
    return mybir.dt.float32r, mybir.dt.float32r


def _build(t_steps=T):
    import concourse.tile as tile
    from concourse import bacc, mybir

    f32 = mybir.dt.float32
    MMD, EVD = _dts()
    AF = mybir.ActivationFunctionType
    OP = mybir.AluOpType

    nc = bacc.Bacc("TRN2", target_bir_lowering=False, debug=False,
                   num_devices=NCORES)

    xT = nc.dram_tensor("xT", [t_steps, 128, KI, BS], MMD,
                        kind="ExternalInput").ap()
    h0T = nc.dram_tensor("h0T", [128, KH, BS], MMD, kind="ExternalInput").ap()
    wat1 = nc.dram_tensor("wat1", [128, KH, IP], MMD, kind="ExternalInput").ap()
    wat2 = nc.dram_tensor("wat2", [128, KI, IP], MMD, kind="ExternalInput").ap()
    wat3 = nc.dram_tensor("wat3", [128, KI, IP], MMD, kind="ExternalInput").ap()
    wih = nc.dram_tensor("wih", [128, KI, G], MMD, kind="ExternalInput").ap()
    whh = nc.dram_tensor("whh", [128, KH, G], MMD, kind="ExternalInput").ap()
    onesw = nc.dram_tensor("onesw", [128, 128], EVD, kind="ExternalInput").ap()
    bias_u_d = nc.dram_tensor("bias_u", [128, KI], f32, kind="ExternalInput").ap()
    bias_v_d = nc.dram_tensor("bias_v", [128, KI], f32, kind="ExternalInput").ap()
    # rz bias pre-halved for the tanh-based sigmoid
    bias_rzh_d = nc.dram_tensor("bias_rzh", [128, 4], f32,
                                kind="ExternalInput").ap()
    bias_hn_d = nc.dram_tensor("bias_hn", [128, 2], f32, kind="ExternalInput").ap()
    bias_in_d = nc.dram_tensor("bias_in", [128, 2], f32, kind="ExternalInput").ap()
    outT = nc.dram_tensor("outT", [t_steps, 128, KH, BS], MMD,
                          kind="ExternalOutput").ap()

    def fv(ap):
        # readable view for DVE of matmul-dtype tiles
        if DT == "f32r":
            return ap.bitcast(f32)
        return ap

    with tile.TileContext(nc) as tc:
        with tc.tile_pool(name="const", bufs=1) as cp, \
             tc.tile_pool(name="xs", bufs=1) as xp, \
             tc.tile_pool(name="hs", bufs=1) as hp, \
             tc.tile_pool(name="wk", bufs=1) as wp, \
             tc.tile_pool(name="ps", bufs=1, space="PSUM") as pp:

            w1t = cp.tile([128, KH, IP], MMD)
            w2t = cp.tile([128, KI, IP], MMD)
            w3t = cp.tile([128, KI, IP], MMD)
            wiht = cp.tile([128, KI, G], MMD)
            whht = cp.tile([128, KH, G], MMD)
            onest = cp.tile([128, 128], EVD)
            bu = cp.tile([128, KI], f32)
            bv = cp.tile([128, KI], f32)
            brzh = cp.tile([128, 4], f32)
            bhn = cp.tile([128, 2], f32)
            bin_ = cp.tile([128, 2], f32)
            # h0 + step-0 x first (they gate the first matmuls), then
            # weights ordered by first use, alternating the two HWDGE rings
            hcur = []
            for ci in range(NCHUNK):
                hc = hp.tile([128, KH, CB], MMD, name=f"h_{ci}",
                             tag=f"h{ci}", bufs=2)
                nc.scalar.dma_start(
                    out=hc[:], in_=h0T[:, :, ci * CB:(ci + 1) * CB])
                hcur.append(hc)
            x_pre = xp.tile([128, KI, BS], MMD, name="x_pre", tag="x", bufs=4)
            nc.sync.dma_start(out=x_pre[:], in_=xT[0])
            for i, (dst, src) in enumerate([
                    (w2t, wat2), (w1t, wat1), (bu, bias_u_d),
                    (w3t, wat3), (bv, bias_v_d), (onest, onesw),
                    (whht, whh), (wiht, wih),
                    (brzh, bias_rzh_d), (bhn, bias_hn_d),
                    (bin_, bias_in_d)]):
                eng = nc.sync if i % 2 == 0 else nc.scalar
                eng.dma_start(out=dst[:], in_=src)

            def ms(m):
                return slice(m * 128, (m + 1) * 128)

            for t in range(t_steps):
                if t == 0:
                    x_t = x_pre
                else:
                    x_t = xp.tile([128, KI, BS], MMD, name=f"x_{t}",
                                  tag="x", bufs=4)
                    nc.sync.dma_start(out=x_t[:], in_=xT[t])

                st = [{} for _ in range(NCHUNK)]

                # ---- phase 1: h-gate matmuls + attention stage 1 ----
                for ci in range(NCHUNK):
                    cs = slice(ci * CB, (ci + 1) * CB)
                    h = hcur[ci]
                    ps_u = [pp.tile([128, CB], f32,
                                    name=f"psu{m}_{t}_{ci}", tag="aps",
                                    bufs=4) for m in range(KI)]
                    for m in range(KI):
                        for k in range(KI):
                            nc.tensor.matmul(
                                ps_u[m][:], w2t[:, k, ms(m)],
                                x_t[:, k, cs], start=(k == 0), stop=False)
                        for k in range(KH):
                            nc.tensor.matmul(
                                ps_u[m][:], w1t[:, k, ms(m)],
                                h[:, k, :], start=False, stop=(k == KH - 1))
                    u = wp.tile([128, KI, CB], MMD, name=f"u_{t}_{ci}",
                                tag="u", bufs=3)
                    for m in range(KI):
                        nc.scalar.activation(u[:, m, :], ps_u[m][:],
                                             AF.Tanh, bias=bu[:, m:m + 1])
                    st[ci].update(u=u)

                # ---- phase 2: v, softmax, wx ----
                for ci in range(NCHUNK):
                    cs = slice(ci * CB, (ci + 1) * CB)
                    u = st[ci]["u"]
                    ps_v = [pp.tile([128, CB], f32,
                                    name=f"psv{m}_{t}_{ci}", tag="aps",
                                    bufs=4) for m in range(KI)]
                    for m in range(KI):
                        for k in range(KI):
                            nc.tensor.matmul(
                                ps_v[m][:], w3t[:, k, ms(m)],
                                u[:, k, :], start=(k == 0), stop=(k == KI - 1))
                    ev = wp.tile([128, KI, CB], EVD, name=f"ev_{t}_{ci}",
                                 tag="ev", bufs=3)
                    for m in range(KI):
                        nc.scalar.activation(ev[:, m, :], ps_v[m][:],
                                             AF.Exp, bias=bv[:, m:m + 1])
                    ps_den = pp.tile([128, CB], f32, name=f"psden_{t}_{ci}",
                                     tag="aps", bufs=4)
                    for k in range(KI):
                        nc.tensor.matmul(ps_den[:], onest[:], ev[:, k, :],
                                         start=(k == 0), stop=(k == KI - 1))
                    rinv = wp.tile([128, CB], f32, name=f"rinv_{t}_{ci}",
                                   tag="rinv", bufs=3)
                    nc.vector.reciprocal_approx_fast(rinv[:], ps_den[:])
                    rinv16 = wp.tile([128, CB], MMD, name=f"rinv16_{t}_{ci}",
                                     tag="rinv16", bufs=3)
                    nc.vector.tensor_copy(rinv16[:], rinv[:])
                    wx = wp.tile([128, KI, CB], MMD, name=f"wx_{t}_{ci}",
                                 tag="wx", bufs=3)
                    nc.vector.tensor_mul(wx[:], fv(x_t[:, :, cs]), fv(ev[:]))
                    _r = rinv16[:]
                    nc.vector.tensor_mul(wx[:, 0, :], fv(wx[:, 0, :]), _r)
                    rrep = bass.AP(tensor=_r.tensor, offset=_r.offset,
                                   ap=[_r.ap[0], [0, KI - 1], _r.ap[1]])
                    nc.vector.tensor_mul(wx[:, 1:KI, :], fv(wx[:, 1:KI, :]),
                                         rrep)
                    st[ci].update(wx=wx)

                # ---- phase 3: gate matmuls + GRU tail ----
                for ci in range(NCHUNK):
                    cs = slice(ci * CB, (ci + 1) * CB)
                    h = hcur[ci]
                    wx = st[ci]["wx"]
                    ps_hn = pp.tile([128, 2, CB], f32, name=f"pshn_{t}_{ci}",
                                    tag="gps", bufs=4)
                    for m in range(2):
                        for k in range(KH):
                            nc.tensor.matmul(
                                ps_hn[:, m, :], whht[:, k, ms(4 + m)],
                                h[:, k, :], start=(k == 0), stop=(k == KH - 1))
                    ps_r = pp.tile([128, 2, CB], f32, name=f"psr_{t}_{ci}",
                                   tag="gps", bufs=4)
                    ps_z = pp.tile([128, 2, CB], f32, name=f"psz_{t}_{ci}",
                                   tag="gps", bufs=4)
                    # h-only whh matmuls of the m0 slices first (r and z are
                    # different banks, so both groups may be open at once):
                    # they keep the in-order PE stream fed while wx lands
                    for mm_t, base in ((ps_r, 0), (ps_z, 2)):
                        for k in range(KH):
                            nc.tensor.matmul(
                                mm_t[:, 0, :], whht[:, k, ms(base)],
                                h[:, k, :], start=(k == 0), stop=False)
                    for mm_t, base in ((ps_r, 0), (ps_z, 2)):
                        for k in range(KI):
                            nc.tensor.matmul(
                                mm_t[:, 0, :], wiht[:, k, ms(base)],
                                wx[:, k, :], start=False, stop=(k == KI - 1))
                        for k in range(KH):
                            nc.tensor.matmul(
                                mm_t[:, 1, :], whht[:, k, ms(base + 1)],
                                h[:, k, :], start=(k == 0), stop=False)
                        for k in range(KI):
                            nc.tensor.matmul(
                                mm_t[:, 1, :], wiht[:, k, ms(base + 1)],
                                wx[:, k, :], start=False, stop=(k == KI - 1))
                    ps_in = pp.tile([128, 2, CB], f32, name=f"psin_{t}_{ci}",
                                    tag="gps", bufs=4)
                    for m in range(2):
                        for k in range(KI):
                            nc.tensor.matmul(
                                ps_in[:, m, :], wiht[:, k, ms(4 + m)],
                                wx[:, k, :], start=(k == 0), stop=(k == KI - 1))

                    g = wp.tile([128, 4, CB], MMD, name=f"g_{t}_{ci}",
                                tag="g", bufs=3)
                    for m in range(4):
                        src_ps = ps_r if m < 2 else ps_z
                        nc.scalar.activation(g[:, m, :], src_ps[:, m % 2, :],
                                             AF.Tanh, bias=brzh[:, m:m + 1],
                                             scale=0.5)
                    t1h = wp.tile([128, 2, CB], MMD, name=f"t1h_{t}_{ci}",
                                  tag="t1h", bufs=3)
                    for m in range(2):
                        nc.vector.tensor_scalar(
                            out=t1h[:, m, :], in0=ps_hn[:, m, :],
                            scalar1=bhn[:, m:m + 1], scalar2=0.5,
                            op0=OP.add, op1=OP.mult)
                    # p = (i_n + b_in) + t1h is g-independent: compute it
                    # early so only two fp16 DVE ops trail the gate ACT
                    p_ = wp.tile([128, 2, CB], MMD, name=f"p_{t}_{ci}",
                                 tag="p", bufs=3)
                    for m in range(2):
                        nc.vector.scalar_tensor_tensor(
                            p_[:, m, :], ps_in[:, m, :], bin_[:, m:m + 1],
                            t1h[:, m, :], OP.add, OP.add)
                    t0h = wp.tile([128, 2, CB], MMD, name=f"t0h_{t}_{ci}",
                                  tag="t0h", bufs=3)
                    nc.vector.tensor_mul(t0h[:], t1h[:], g[:, 0:2, :])
                    s2 = wp.tile([128, 2, CB], MMD, name=f"s2_{t}_{ci}",
                                 tag="s2", bufs=3)
                    nc.vector.tensor_add(s2[:], t0h[:], p_[:])
                    n = wp.tile([128, 2, CB], MMD, name=f"n_{t}_{ci}",
                                tag="n", bufs=3)
                    nc.scalar.activation(n[:], s2[:], AF.Tanh)

                    zz = wp.tile([128, 2, CB], MMD, name=f"zz_{t}_{ci}",
                                 tag="zz", bufs=3)
                    nc.vector.tensor_scalar(
                        out=zz[:], in0=g[:, 2:4, :], scalar1=0.5, scalar2=0.5,
                        op0=OP.mult, op1=OP.add)
                    w1z = wp.tile([128, 2, CB], MMD, name=f"w1z_{t}_{ci}",
                                  tag="w1z", bufs=3)
                    nc.vector.tensor_scalar(
                        out=w1z[:], in0=g[:, 2:4, :], scalar1=-0.5,
                        scalar2=0.5, op0=OP.mult, op1=OP.add)
                    bzh = wp.tile([128, 2, CB], MMD, name=f"bzh_{t}_{ci}",
                                  tag="bzh", bufs=3)
                    nc.vector.tensor_mul(bzh[:], zz[:], fv(h[:]))
                    a4 = wp.tile([128, 2, CB], MMD, name=f"a4_{t}_{ci}",
                                 tag="a4", bufs=3)
                    nc.vector.tensor_mul(a4[:], w1z[:], n[:])
                    h_new = hp.tile([128, KH, CB], MMD, name=f"hn_{t}_{ci}",
                                    tag=f"h{ci}", bufs=2)
                    nc.vector.tensor_add(h_new[:], a4[:], bzh[:])
                    hcur[ci] = h_new

                    nc.sync.dma_start(out=outT[t][:, :, cs], in_=h_new[:])

    nc.compile()
    return nc


# ---------------- host-side data prep ----------------

def _prep_core_inputs(x, h0, attn1_W, attn1_b, attn2_W, attn2_b, attn3_W,
                      attn3_b, W_ih, b_ih, W_hh, b_hh, t_steps=T):
    f4 = np.float32
    MMD, EVD = _dts()
    mnp = _np_dt(MMD)
    enp = _np_dt(EVD)
    x = np.asarray(x, f4)
    h0 = np.asarray(h0, f4)

    A1 = np.asarray(attn1_W, f4)                       # [I, H]
    w1 = np.zeros((H, IP), f4)
    w1[:, :I] = A1.T                                   # lhsT[hh, ii]
    wat1 = np.ascontiguousarray(
        w1.reshape(KH, 128, IP).transpose(1, 0, 2)).astype(mnp)

    A2 = np.asarray(attn2_W, f4)                       # [I, I] (out, in)
    w2 = np.zeros((IP, IP), f4)
    w2[:I, :I] = A2.T                                  # lhsT[in, out]
    wat2 = np.ascontiguousarray(
        w2.reshape(KI, 128, IP).transpose(1, 0, 2)).astype(mnp)

    A3 = np.asarray(attn3_W, f4)
    w3 = np.zeros((IP, IP), f4)
    w3[:I, :I] = A3.T
    wat3 = np.ascontiguousarray(
        w3.reshape(KI, 128, IP).transpose(1, 0, 2)).astype(mnp)

    Wi = np.asarray(W_ih, f4)                          # [G, I]
    wi = np.zeros((IP, G), f4)
    wi[:I, :] = Wi.T
    wih = np.ascontiguousarray(
        wi.reshape(KI, 128, G).transpose(1, 0, 2)).astype(mnp)

    Wh = np.asarray(W_hh, f4)                          # [G, H]
    whh = np.ascontiguousarray(
        Wh.T.reshape(KH, 128, G).transpose(1, 0, 2)).astype(mnp)

    onesw = np.ones((128, 128), enp)

    bu = np.zeros(IP, f4)
    bu[:I] = np.asarray(attn1_b, f4) + np.asarray(attn2_b, f4)
    bias_u = np.ascontiguousarray(bu.reshape(KI, 128).T)
    bvv = np.full(IP, -1e4, f4)
    bvv[:I] = np.asarray(attn3_b, f4)
    bias_v = np.ascontiguousarray(bvv.reshape(KI, 128).T)
    brz = (np.asarray(b_ih, f4) + np.asarray(b_hh, f4))[:2 * H] * 0.5
    bias_rzh = np.ascontiguousarray(brz.reshape(4, 128).T)
    bias_hn = np.ascontiguousarray(
        np.asarray(b_hh, f4)[2 * H:].reshape(2, 128).T)
    bias_in = np.ascontiguousarray(
        np.asarray(b_ih, f4)[2 * H:].reshape(2, 128).T)

    x16 = x[:, :t_steps, :].astype(mnp)
    xpad = np.pad(x16, ((0, 0), (0, 0), (0, IP - I)))
    # [NC, BS, T, KI, 128] -> [NC, T, 128, KI, BS]
    xr = xpad.reshape(NCORES, BS, t_steps, KI, 128).transpose(0, 2, 4, 3, 1)
    h0r = h0.astype(mnp).reshape(NCORES, BS, KH, 128).transpose(0, 3, 2, 1)

    shared = dict(wat1=wat1, wat2=wat2, wat3=wat3, wih=wih, whh=whh,
                  onesw=onesw, bias_u=bias_u, bias_v=bias_v,
                  bias_rzh=bias_rzh, bias_hn=bias_hn, bias_in=bias_in)
    in_maps = []
    for c in range(NCORES):
        m = dict(shared)
        m["xT"] = np.ascontiguousarray(xr[c])
        m["h0T"] = np.ascontiguousarray(h0r[c])
        in_maps.append(m)
    return in_maps


def _gather(results, t_steps=T):
    outs = []
    for c in range(NCORES):
        o = np.asarray(results[c]["outT"], np.float32)
        outs.append(o.transpose(3, 0, 2, 1).reshape(BS, t_steps, H))
    return np.ascontiguousarray(np.concatenate(outs, axis=0))


def _get_nc(t_steps=T):
    key = ("nc", t_steps, DT)
    if key not in _STATE:
        _STATE[key] = _build(t_steps)
    return _STATE[key]


def run(inputs, trace=False, t_steps=T):
    from concourse.bass_utils import run_bass_kernel_spmd
    nc = _get_nc(t_steps)
    in_maps = _prep_core_inputs(t_steps=t_steps, **inputs)
    res = run_bass_kernel_spmd(nc, in_maps, list(range(NCORES)), trace=trace)
    return _gather(res.results, t_steps), res


def kernel(**inputs):
    out, _ = run(inputs, trace=False)
    return out


# revision 26
# speedup vs baseline: 1.0997x; 1.0997x over previous
"""Trainium2 Bass kernel for nn_Encoder_81595788689580.

Attention-gated GRU encoder: per time step
    w1 = h @ attn1_W.T + attn1_b
    w2 = x_t @ attn2_W.T + attn2_b
    v  = tanh(w1 + w2) @ attn3_W.T + attn3_b
    alpha = softmax(v, axis=feature)
    wx = x_t * alpha
    GRU cell (r, z, n) -> h_new
Output: [B, T, H] hidden states.

Strategy (8 NeuronCores, data-parallel over batch):
  - batch 4096 -> 512 rows per core; all weights replicated.
  - everything stored TRANSPOSED on chip: features on partitions, batch on
    the free dim. Every matmul is weights-stationary with batch as the
    moving dim, biases become per-partition ACT bias vectors, and no
    transposes are ever needed on device (host pre-/post-transposes).
  - feature dim I=320 zero-padded to 384 = 3x128 partition blocks; padded
    attn3_b rows are -1e4 so exp() of pad rows is exactly 0 and the
    softmax denominator is unaffected.
  - softmax over features is a partition reduction: an all-ones stationary
    matmul broadcasts the per-column denominator into all 128 partitions
    of one PSUM tile; max-subtraction is skipped (|v| <= ~8 in practice,
    exp stays finite, softmax is shift-invariant).
  - sigmoid is computed as 0.5*tanh(x/2)+0.5 so every ACT op uses the
    exp_and_others table set -- avoids ~2.7us ACT table swaps per step.
  - matmuls in fp16 (1 PE cycle/row, fast weight loads) with fp32 PSUM
    accumulation; exp output in bf16 (range safety). DT="f32r" switches
    to float32r matmuls (~10x lower error, ~1.7x slower weight loads).
  - the 512-row batch runs as 2 independent chunks of 256 so the two
    recurrences pipeline against each other across engines.
"""

import numpy as np

B, T, I, H = 4096, 24, 320, 256
NCORES = 8
BS = B // NCORES          # 512 rows per core
IP = 384                  # I padded to 3*128
KI = IP // 128            # 3 feature blocks
KH = H // 128             # 2 hidden blocks
G = 3 * H                 # 768 gate rows
NCHUNK = 2
CB = BS // NCHUNK         # 256 batch columns per chunk

DT = "f16"                # "f16" | "f32r"

_STATE = {}


def _np_dt(mdt):
    from concourse import mybir
    return mybir.dt.np(mdt)


def _dts():
    from concourse import mybir
    if DT == "f16":
        return mybir.dt.float16, mybir.dt.bfloat16
    return mybir.dt.float32r, mybir.dt.float32r


def _build(t_steps=T):
    import concourse.tile as tile
    from concourse import bacc, mybir

    f32 = mybir.dt.float32
    MMD, EVD = _dts()
    AF = mybir.ActivationFunctionType
    OP = mybir.AluOpType

    nc = bacc.Bacc("TRN2", target_bir_lowering=False, debug=False,
                   num_devices=NCORES)

    xT = nc.dram_tensor("xT", [t_steps, 128, KI, BS], MMD,
                        kind="ExternalInput").ap()
    h0T = nc.dram_tensor("h0T", [128, KH, BS], MMD, kind="ExternalInput").ap()
    wat1 = nc.dram_tensor("wat1", [128, KH, IP], MMD, kind="ExternalInput").ap()
    wat2 = nc.dram_tensor("wat2", [128, KI, IP], MMD, kind="ExternalInput").ap()
    wat3 = nc.dram_tensor("wat3", [128, KI, IP], MMD, kind="ExternalInput").ap()
    wih = nc.dram_tensor("wih", [128, KI, G], MMD, kind="ExternalInput").ap()
    whh = nc.dram_tensor("whh", [128, KH, G], MMD, kind="ExternalInput").ap()
    onesw = nc.dram_tensor("onesw", [128, 128], EVD, kind="ExternalInput").ap()
    bias_u_d = nc.dram_tensor("bias_u", [128, KI], f32, kind="ExternalInput").ap()
    bias_v_d = nc.dram_tensor("bias_v", [128, KI], f32, kind="ExternalInput").ap()
    # rz bias pre-halved for the tanh-based sigmoid
    bias_rzh_d = nc.dram_tensor("bias_rzh", [128, 4], f32,
                                kind="ExternalInput").ap()
    bias_hn_d = nc.dram_tensor("bias_hn", [128, 2], f32, kind="ExternalInput").ap()
    bias_in_d = nc.dram_tensor("bias_in", [128, 2], f32, kind="ExternalInput").ap()
    outT = nc.dram_tensor("outT", [t_steps, 128, KH, BS], MMD,
                          kind="ExternalOutput").ap()

    def fv(ap):
        # readable view for DVE of matmul-dtype tiles
        if DT == "f32r":
            return ap.bitcast(f32)
        return ap

    with tile.TileContext(nc) as tc:
        with tc.tile_pool(name="const", bufs=1) as cp, \
             tc.tile_pool(name="xs", bufs=1) as xp, \
             tc.tile_pool(name="hs", bufs=1) as hp, \
             tc.tile_pool(name="wk", bufs=1) as wp, \
             tc.tile_pool(name="ps", bufs=1, space="PSUM") as pp:

            w1t = cp.tile([128, KH, IP], MMD)
            w2t = cp.tile([128, KI, IP], MMD)
            w3t = cp.tile([128, KI, IP], MMD)
            wiht = cp.tile([128, KI, G], MMD)
            whht = cp.tile([128, KH, G], MMD)
            onest = cp.tile([128, 128], EVD)
            bu = cp.tile([128, KI], f32)
            bv = cp.tile([128, KI], f32)
            brzh = cp.tile([128, 4], f32)
            bhn = cp.tile([128, 2], f32)
            bin_ = cp.tile([128, 2], f32)
            # h0 + step-0 x first (they gate the first matmuls), then
            # weights ordered by first use, alternating the two HWDGE rings
            hcur = []
            for ci in range(NCHUNK):
                hc = hp.tile([128, KH, CB], MMD, name=f"h_{ci}",
                             tag=f"h{ci}", bufs=2)
                nc.scalar.dma_start(
                    out=hc[:], in_=h0T[:, :, ci * CB:(ci + 1) * CB])
                hcur.append(hc)
            x_pre = xp.tile([128, KI, BS], MMD, name="x_pre", tag="x", bufs=4)
            nc.sync.dma_start(out=x_pre[:], in_=xT[0])
            for i, (dst, src) in enumerate([
                    (w2t, wat2), (w1t, wat1), (bu, bias_u_d),
                    (w3t, wat3), (bv, bias_v_d), (onest, onesw),
                    (whht, whh), (wiht, wih),
                    (brzh, bias_rzh_d), (bhn, bias_hn_d),
                    (bin_, bias_in_d)]):
                eng = nc.sync if i % 2 == 0 else nc.scalar
                eng.dma_start(out=dst[:], in_=src)

            def ms(m):
                return slice(m * 128, (m + 1) * 128)

            for t in range(t_steps):
                if t == 0:
                    x_t = x_pre
                else:
                    x_t = xp.tile([128, KI, BS], MMD, name=f"x_{t}",
                                  tag="x", bufs=4)
                    nc.sync.dma_start(out=x_t[:], in_=xT[t])

                st = [{} for _ in range(NCHUNK)]

                # ---- phase 1: h-gate matmuls + attention stage 1 ----
                for ci in range(NCHUNK):
                    cs = slice(ci * CB, (ci + 1) * CB)
                    h = hcur[ci]
                    ps_u = [pp.tile([128, CB], f32,
                                    name=f"psu{m}_{t}_{ci}", tag="aps",
                                    bufs=5) for m in range(KI)]
                    for m in range(KI):
                        for k in range(KI):
                            nc.tensor.matmul(
                                ps_u[m][:], w2t[:, k, ms(m)],
                                x_t[:, k, cs], start=(k == 0), stop=False)
                        for k in range(KH):
                            nc.tensor.matmul(
                                ps_u[m][:], w1t[:, k, ms(m)],
                                h[:, k, :], start=False, stop=(k == KH - 1))
                    u = wp.tile([128, KI, CB], MMD, name=f"u_{t}_{ci}",
                                tag="u", bufs=3)
                    for m in range(KI):
                        nc.scalar.activation(u[:, m, :], ps_u[m][:],
                                             AF.Tanh, bias=bu[:, m:m + 1])
                    st[ci].update(u=u)

                # ---- phase 2: v, softmax, wx ----
                for ci in range(NCHUNK):
                    cs = slice(ci * CB, (ci + 1) * CB)
                    u = st[ci]["u"]
                    ps_v = [pp.tile([128, CB], f32,
                                    name=f"psv{m}_{t}_{ci}", tag="aps",
                                    bufs=5) for m in range(KI)]
                    for m in range(KI):
                        for k in range(KI):
                            nc.tensor.matmul(
                                ps_v[m][:], w3t[:, k, ms(m)],
                                u[:, k, :], start=(k == 0), stop=(k == KI - 1))
                    ev = wp.tile([128, KI, CB], EVD, name=f"ev_{t}_{ci}",
                                 tag="ev", bufs=3)
                    for m in range(KI):
                        nc.scalar.activation(ev[:, m, :], ps_v[m][:],
                                             AF.Exp, bias=bv[:, m:m + 1])
                    ps_den = pp.tile([128, CB], f32, name=f"psden_{t}_{ci}",
                                     tag="aps", bufs=5)
                    for k in range(KI):
                        nc.tensor.matmul(ps_den[:], onest[:], ev[:, k, :],
                                         start=(k == 0), stop=(k == KI - 1))
                    rinv = wp.tile([128, CB], f32, name=f"rinv_{t}_{ci}",
                                   tag="rinv", bufs=3)
                    nc.vector.reciprocal_approx_fast(rinv[:], ps_den[:])
                    rinv16 = wp.tile([128, CB], MMD, name=f"rinv16_{t}_{ci}",
                                     tag="rinv16", bufs=3)
                    nc.vector.tensor_copy(rinv16[:], rinv[:])
                    wx = wp.tile([128, KI, CB], MMD, name=f"wx_{t}_{ci}",
                                 tag="wx", bufs=3)
                    nc.vector.tensor_mul(wx[:], fv(x_t[:, :, cs]), fv(ev[:]))
                    _r = rinv16[:]
                    nc.vector.tensor_mul(wx[:, 0, :], fv(wx[:, 0, :]), _r)
                    rrep = bass.AP(tensor=_r.tensor, offset=_r.offset,
                                   ap=[_r.ap[0], [0, KI - 1], _r.ap[1]])
                    nc.vector.tensor_mul(wx[:, 1:KI, :], fv(wx[:, 1:KI, :]),
                                         rrep)
                    st[ci].update(wx=wx)

                # ---- phase 3: gate matmuls + GRU tail ----
                for ci in range(NCHUNK):
                    cs = slice(ci * CB, (ci + 1) * CB)
                    h = hcur[ci]
                    wx = st[ci]["wx"]
                    ps_hn = pp.tile([128, 2, CB], f32, name=f"pshn_{t}_{ci}",
                                    tag="gps", bufs=3)
                    for m in range(2):
                        for k in range(KH):
                            nc.tensor.matmul(
                                ps_hn[:, m, :], whht[:, k, ms(4 + m)],
                                h[:, k, :], start=(k == 0), stop=(k == KH - 1))
                    ps_r = pp.tile([128, 2, CB], f32, name=f"psr_{t}_{ci}",
                                   tag="gps", bufs=3)
                    ps_z = pp.tile([128, 2, CB], f32, name=f"psz_{t}_{ci}",
                                   tag="gps", bufs=3)
                    # h-only whh matmuls of the m0 slices first (r and z are
                    # different banks, so both groups may be open at once):
                    # they keep the in-order PE stream fed while wx lands
                    for mm_t, base in ((ps_r, 0), (ps_z, 2)):
                        for k in range(KH):
                            nc.tensor.matmul(
                                mm_t[:, 0, :], whht[:, k, ms(base)],
                                h[:, k, :], start=(k == 0), stop=False)
                    for mm_t, base in ((ps_r, 0), (ps_z, 2)):
                        for k in range(KI):
                            nc.tensor.matmul(
                                mm_t[:, 0, :], wiht[:, k, ms(base)],
                                wx[:, k, :], start=False, stop=(k == KI - 1))
                        for k in range(KH):
                            nc.tensor.matmul(
                                mm_t[:, 1, :], whht[:, k, ms(base + 1)],
                                h[:, k, :], start=(k == 0), stop=False)
                        for k in range(KI):
                            nc.tensor.matmul(
                                mm_t[:, 1, :], wiht[:, k, ms(base + 1)],
                                wx[:, k, :], start=False, stop=(k == KI - 1))
                    ps_in = pp.tile([128, 2, CB], f32, name=f"psin_{t}_{ci}",
                                    tag="gps", bufs=3)
                    for m in range(2):
                        for k in range(KI):
                            nc.tensor.matmul(
                                ps_in[:, m, :], wiht[:, k, ms(4 + m)],
                                wx[:, k, :], start=(k == 0), stop=(k == KI - 1))

                    g = wp.tile([128, 4, CB], MMD, name=f"g_{t}_{ci}",
                                tag="g", bufs=3)
                    for m in range(4):
                        src_ps = ps_r if m < 2 else ps_z
                        nc.scalar.activation(g[:, m, :], src_ps[:, m % 2, :],
                                             AF.Tanh, bias=brzh[:, m:m + 1],
                                             scale=0.5)
                    t1h = wp.tile([128, 2, CB], MMD, name=f"t1h_{t}_{ci}",
                                  tag="t1h", bufs=3)
                    for m in range(2):
                        nc.vector.tensor_scalar(
                            out=t1h[:, m, :], in0=ps_hn[:, m, :],
                            scalar1=bhn[:, m:m + 1], scalar2=0.5,
                            op0=OP.add, op1=OP.mult)
                    # p = (i_n + b_in) + t1h is g-independent: compute it
                    # early so only two fp16 DVE ops trail the gate ACT
                    p_ = wp.tile([128, 2, CB], MMD, name=f"p_{t}_{ci}",
                                 tag="p", bufs=3)
                    for m in range(2):
                        nc.vector.scalar_tensor_tensor(
                            p_[:, m, :], ps_in[:, m, :], bin_[:, m:m + 1],
                            t1h[:, m, :], OP.add, OP.add)
                    t0h = wp.tile([128, 2, CB], MMD, name=f"t0h_{t}_{ci}",
                                  tag="t0h", bufs=3)
                    nc.vector.tensor_mul(t0h[:], t1h[:], g[:, 0:2, :])
                    s2 = wp.tile([128, 2, CB], MMD, name=f"s2_{t}_{ci}",
                                 tag="s2", bufs=3)
                    nc.vector.tensor_add(s2[:], t0h[:], p_[:])
                    n = wp.tile([128, 2, CB], MMD, name=f"n_{t}_{ci}",
                                tag="n", bufs=3)
                    nc.scalar.activation(n[:], s2[:], AF.Tanh)

                    zz = wp.tile([128, 2, CB], MMD, name=f"zz_{t}_{ci}",
                                 tag="zz", bufs=3)
                    nc.vector.tensor_scalar(
                        out=zz[:], in0=g[:, 2:4, :], scalar1=0.5, scalar2=0.5,
                        op0=OP.mult, op1=OP.add)
                    w1z = wp.tile([128, 2, CB], MMD, name=f"w1z_{t}_{ci}",
                                  tag="w1z", bufs=3)
                    nc.vector.tensor_scalar(
                        out=w1z[:], in0=g[:, 2:4, :], scalar1=-0.5,
                        scalar2=0.5, op0=OP.mult, op1=OP.add)
                    bzh = wp.tile([128, 2, CB], MMD, name=f"bzh_{t}_{ci}",
                                  tag="bzh", bufs=3)
                    nc.vector.tensor_mul(bzh[:], zz[:], fv(h[:]))
                    a4 = wp.tile([128, 2, CB], MMD, name=f"a4_{t}_{ci}",
                                 tag="a4", bufs=3)
                    nc.vector.tensor_mul(a4[:], w1z[:], n[:])
                    h_new = hp.tile([128, KH, CB], MMD, name=f"hn_{t}_{ci}",
                                    tag=f"h{ci}", bufs=2)
                    nc.vector.tensor_add(h_new[:], a4[:], bzh[:])
                    hcur[ci] = h_new

                    nc.sync.dma_start(out=outT[t][:, :, cs], in_=h_new[:])

    nc.compile()
    return nc


# ---------------- host-side data prep ----------------

def _prep_core_inputs(x, h0, attn1_W, attn1_b, attn2_W, attn2_b, attn3_W,
                      attn3_b, W_ih, b_ih, W_hh, b_hh, t_steps=T):
    f4 = np.float32
    MMD, EVD = _dts()
    mnp = _np_dt(MMD)
    enp = _np_dt(EVD)
    x = np.asarray(x, f4)
    h0 = np.asarray(h0, f4)

    A1 = np.asarray(attn1_W, f4)                       # [I, H]
    w1 = np.zeros((H, IP), f4)
    w1[:, :I] = A1.T                                   # lhsT[hh, ii]
    wat1 = np.ascontiguousarray(
        w1.reshape(KH, 128, IP).transpose(1, 0, 2)).astype(mnp)

    A2 = np.asarray(attn2_W, f4)                       # [I, I] (out, in)
    w2 = np.zeros((IP, IP), f4)
    w2[:I, :I] = A2.T                                  # lhsT[in, out]
    wat2 = np.ascontiguousarray(
        w2.reshape(KI, 128, IP).transpose(1, 0, 2)).astype(mnp)

    A3 = np.asarray(attn3_W, f4)
    w3 = np.zeros((IP, IP), f4)
    w3[:I, :I] = A3.T
    wat3 = np.ascontiguousarray(
        w3.reshape(KI, 128, IP).transpose(1, 0, 2)).astype(mnp)

    Wi = np.asarray(W_ih, f4)                          # [G, I]
    wi = np.zeros((IP, G), f4)
    wi[:I, :] = Wi.T
    wih = np.ascontiguousarray(
        wi.reshape(KI, 128, G).transpose(1, 0, 2)).astype(mnp)

    Wh = np.asarray(W_hh, f4)                          # [G, H]
    whh = np.ascontiguousarray(
        Wh.T.reshape(KH, 128, G).transpose(1, 0, 2)).astype(mnp)

    onesw = np.ones((128, 128), enp)

    bu = np.zeros(IP, f4)
    bu[:I] = np.asarray(attn1_b, f4) + np.asarray(attn2_b, f4)
    bias_u = np.ascontiguousarray(bu.reshape(KI, 128).T)
    bvv = np.full(IP, -1e4, f4)
    bvv[:I] = np.asarray(attn3_b, f4)
    bias_v = np.ascontiguousarray(bvv.reshape(KI, 128).T)
    brz = (np.asarray(b_ih, f4) + np.asarray(b_hh, f4))[:2 * H] * 0.5
    bias_rzh = np.ascontiguousarray(brz.reshape(4, 128).T)
    bias_hn = np.ascontiguousarray(
        np.asarray(b_hh, f4)[2 * H:].reshape(2, 128).T)
    bias_in = np.ascontiguousarray(
        np.asarray(b_ih, f4)[2 * H:].reshape(2, 128).T)

    x16 = x[:, :t_steps, :].astype(mnp)
    xpad = np.pad(x16, ((0, 0), (0, 0), (0, IP - I)))
    # [NC, BS, T, KI, 128] -> [NC, T, 128, KI, BS]
    xr = xpad.reshape(NCORES, BS, t_steps, KI, 128).transpose(0, 2, 4, 3, 1)
    h0r = h0.astype(mnp).reshape(NCORES, BS, KH, 128).transpose(0, 3, 2, 1)

    shared = dict(wat1=wat1, wat2=wat2, wat3=wat3, wih=wih, whh=whh,
                  onesw=onesw, bias_u=bias_u, bias_v=bias_v,
                  bias_rzh=bias_rzh, bias_hn=bias_hn, bias_in=bias_in)
    in_maps = []
    for c in range(NCORES):
        m = dict(shared)
        m["xT"] = np.ascontiguousarray(xr[c])
        m["h0T"] = np.ascontiguousarray(h0r[c])
        in_maps.append(m)
    return in_maps


def _gather(results, t_steps=T):
    outs = []
    for c in range(NCORES):
        o = np.asarray(results[c]["outT"], np.float32)
        outs.append(o.transpose(3, 0, 2, 1).reshape(BS, t_steps, H))
    return np.ascontiguousarray(np.concatenate(outs, axis=0))


def _get_nc(t_steps=T):
    key = ("nc", t_steps, DT)
    if key not in _STATE:
        _STATE[key] = _build(t_steps)
    return _STATE[key]


def run(inputs, trace=False, t_steps=T):
    from concourse.bass_utils import run_bass_kernel_spmd
    nc = _get_nc(t_steps)
    in_maps = _prep_core_inputs(t_steps=t_steps, **inputs)
    res = run_bass_kernel_spmd(nc, in_maps, list(range(NCORES)), trace=trace)
    return _gather(res.results, t_steps), res


def kernel(**inputs):
    out, _ = run(inputs, trace=False)
    return out
